# revision 3
# baseline (speedup 1.0000x reference)
"""ArcFace softmax loss on 8 TRN2 NeuronCores (batch-parallel).

512 rows are split 64 rows/core. Each core streams its (64, 100000) f32
shard through ScalarE exp (with free-axis accumulate) at DMA fabric rate,
fixes up the label column per row (from host-gathered c_y =
costh[i, label_i]), and reduces to a partial sum of its per-row losses.
The host unshard step sums the 8 per-core partials.

Math: logits = SCALE*costh with the label column replaced by
SCALE*cos(acos(c_y)+m). Since SCALE*costh <= 63.4, exp cannot overflow
f32, so no max-subtraction pass is needed:
  S_row  = sum_j exp(SCALE*costh[r,j])
  S'_row = S_row - exp(SCALE*c_y) + exp(SCALE*(c_y cos m - sqrt(1-c_y^2) sin m))
  loss   = mean_r( log(S'_row) - SCALE*cos(acos(c_y)+m) )

DMA-engine load balancing (the main trick beyond the baseline):
  Profiles show the 16 per-core DMA engines each take 1/16 of every
  queue's descriptors in contiguous blocks of ceil(n_desc/16), and that
  engine 79 -- the ring host for the dynamic queues -- sustains only
  ~22 GB/s vs ~26.1 GB/s for engines 64-78. With a uniform (128, 50000)
  layout engine 79 (partitions 120-127) finishes ~12-15us after the
  rest. So partitions 120-127 only stream DONOR fewer columns, and the
  displaced data (the last DONOR columns of rows 60-63's stripes) is
  re-routed as a [120, DONOR*8/120] "donor" tile into partitions 0-119:
  a 120-descriptor DMA is dealt to engines 64-78 only (blocks of
  ceil(120/16)=8), so engine 79 never sees it. Same for the "narrow"
  tiles covering columns [FULLW, 50000) of partitions 0-119. Donor data
  is exp-accumulated in its own stats vector and mapped back to rows
  60-63 by a second collapse matrix (edonor), so the per-row sums are
  exact. Net: engine 79 moves 8*(50000-DONOR)*4B at 22 GB/s while
  engines 64-78 move 8*(50000+DONOR/15)*4B at 26.1 GB/s -- both finish
  within ~0.1us of each other instead of 15us apart.

TRN2 specifics that shape the graph:
  - every instruction is arranged to carry at most ONE cross-engine
    dependency (TRN2 engine instructions hold a single semaphore wait):
    each streaming tile has its own SBUF slot (the whole ~202KB/partition
    shard is resident, no WAR/WAW reuse deps), partition reductions run
    as PE matmuls against Pool-built constants, and a zero matmul
    reading the last Pool constant pre-warms PE's vector clock so real
    matmuls only wait on their data input;
  - per-row sums: exp's accum_out gives per-(partition,group) partials
    in stats columns; ACT Copy with accum_out reduces those to
    per-partition totals (same engine as the exps, no cross-engine sem
    hop); PE matmuls with the collapse matrices (emat pairs partitions
    2r,2r+1 into row r; edonor maps donor partition p to row 60+p//30)
    land the row sums in PSUM on top of a preload of delta (the
    label-column fixup);
  - Ln's spline LUT cannot represent inputs ~1e30, so the log runs on
    s * 2^-104 (exact power-of-2 scale in the ACT affine stage) and the
    104*ln2 compensation rides in the accumulated -tn term;
  - one manual ACT table load (natural_log_exp_and_others covers ln,
    exp, copy, identity) so no table switches mid-stream or in the tail;
    sqrt(1-c^2) is computed as exp(0.5*ln(1-c^2)) to stay in that set;
  - streaming tile sizes ramp small->big->small: big tiles keep 20KB
    per-partition DMA descriptors (sustains the ~26GB/s per-engine
    rate); the small lead-in starts ACT ~4us earlier; both the full
    region (engine 79's last work) and the narrow region (engines
    64-78's last work) end in small tiles so the post-last-byte exp is
    sub-microsecond.
"""

import math

import numpy as np

import concourse.bacc as bacc
import concourse.tile as tile
from concourse import mybir
from concourse.bass_utils import run_bass_kernel_spmd
from concourse.hw_specs import get_activation_tables

N_CORES = 8
B, C = 512, 100000
RB = B // N_CORES      # 64 rows per core
HALF = C // 2          # 50000: each row is split into 2 partition stripes

# Donor columns shed from each of partitions 120-127 (engine 79). The
# displaced 8*DONOR elems land as [120, DONOR/15] on partitions 0-119.
DONOR = 7500
DCHUNK = DONOR * 8 // 120       # 500 elems per donor partition
FULLW = HALF - DONOR            # 42500: columns streamed on all 128 partitions

# Full-region tiles (all 128 partitions). Small lead-in, big middle, small
# tail (the tail tile is engine 79's last work).
FULL_TILES = [1250, 5000, 5000, 5000, 5000, 5000, 5000, 5000, 3750, 1750, 750]
assert sum(FULL_TILES) == FULLW
# consecutive FULL_TILES entries per exp instruction
FULL_GROUPS = [1, 2, 2, 2, 1, 1, 1, 1]
assert sum(FULL_GROUPS) == len(FULL_TILES)
# Narrow-region tiles (partitions 0-119 only; engine 79 idle). One exp
# group per tile; the last tile is engines 64-78's last work.
NARROW_TILES = [2500, 2000, 1500, 1000, 500]
assert sum(NARROW_TILES) == DONOR

SCALE = 64.0
MARGIN = 0.5

F32 = mybir.dt.float32
AF = mybir.ActivationFunctionType
ALU = mybir.AluOpType


def _build():
    cos_m = math.cos(MARGIN)
    sin_m = math.sin(MARGIN)

    nc = bacc.Bacc(num_devices=N_CORES)
    costh_ext = nc.declare_dram_parameter("costh", [RB, C], F32, isOutput=False)
    cy_ext = nc.declare_dram_parameter("cy", [RB, 1], F32, isOutput=False)
    out_ext = nc.declare_dram_parameter("out", [1, 1], F32, isOutput=True)

    # (64,100000) viewed as 128 partition stripes: partition 2r+h = row r,
    # class half h. Keeps every DMA partition-dense and contiguous.
    x = costh_ext[:, :].rearrange("r (h c) -> (r h) c", h=2)  # (128, 50000)
    # Donor source: last DONOR columns of rows 60-63's stripes (the data
    # partitions 120-127 do NOT stream), re-striped across 120 partitions.
    donor_src = (costh_ext[60:64, :]
                 .rearrange("r (h c) -> (r h) c", h=2)[:, FULLW:HALF]
                 .rearrange("s (j c) -> s j c", j=15))       # (8, 15, DCHUNK)

    with tile.TileContext(nc) as tc:
        with (
            tc.tile_pool(name="stream", bufs=1) as stream,
            tc.tile_pool(name="small", bufs=1) as small,
            tc.tile_pool(name="psum", bufs=1, space="PSUM") as psum_pool,
        ):
            # ---- Pool-engine constants (built while the first DMAs fly)
            ones = small.tile([RB, 1], F32)
            nc.gpsimd.memset(ones[:, :], 1.0)
            negones = small.tile([RB, 1], F32)
            nc.gpsimd.memset(negones[:, :], -1.0)
            zeros = small.tile([128, 1], F32)
            nc.gpsimd.memset(zeros[:, :], 0.0)
            id64 = small.tile([RB, RB], F32)
            nc.gpsimd.memset(id64[:, :], 0.0)
            nc.gpsimd.affine_select(out=id64[:, :], in_=id64[:, :],
                                    compare_op=ALU.not_equal, fill=1.0, base=0,
                                    pattern=[[-1, RB]], channel_multiplier=1)
            emat = small.tile([128, RB], F32)  # E[p,r] = 1 iff p in {2r, 2r+1}
            nc.gpsimd.memset(emat[:, :], 1.0)
            nc.gpsimd.affine_select(out=emat[:, :], in_=emat[:, :],
                                    compare_op=ALU.is_ge, fill=0.0, base=0,
                                    pattern=[[-2, RB]], channel_multiplier=1)
            nc.gpsimd.affine_select(out=emat[:, :], in_=emat[:, :],
                                    compare_op=ALU.is_ge, fill=0.0, base=1,
                                    pattern=[[2, RB]], channel_multiplier=-1)
            # edonor[p, r] = 1 iff r == 60 + p//30: donor partition p holds a
            # chunk of stripe s=p//15 (= row 60+s//2, half s%2).
            edonor = small.tile([120, RB], F32)
            nc.gpsimd.memset(edonor[:, :], 1.0)
            nc.gpsimd.affine_select(out=edonor[:, :], in_=edonor[:, :],
                                    compare_op=ALU.is_ge, fill=0.0, base=1800,
                                    pattern=[[-30, RB]], channel_multiplier=1)
            nc.gpsimd.affine_select(out=edonor[:, :], in_=edonor[:, :],
                                    compare_op=ALU.is_ge, fill=0.0, base=-1771,
                                    pattern=[[30, RB]], channel_multiplier=-1)

            # One manual ACT table load: natural_log_exp_and_others holds
            # every function this kernel uses (ln, exp, copy, identity), so
            # Bacc's fixpoint inserts no further loads -- not mid-stream, not
            # in the tail before the final Ln.
            _set_names = list(get_activation_tables(nc.m.arch).keys())
            nc.scalar.add_instruction(mybir.InstLoadActFuncSet(
                name=nc.get_next_instruction_name(),
                act_func_set_id=_set_names.index("natural_log_exp_and_others"),
                ins=[], outs=[]))

            # Zero-contribution matmul: initializes the loss accumulator AND
            # (by reading the last-written Pool constant) teaches PE's vector
            # clock about the Pool sem, so later matmuls reading the collapse
            # matrices only need their single data-dependency wait.
            acc_psum = psum_pool.tile([1, 1], F32)
            nc.tensor.matmul(acc_psum[:, :], lhsT=edonor[:, 0:1],
                             rhs=zeros[0:120, :],
                             start=True, stop=False, skip_group_check=True)

            # ---- tiny per-row fixup, depends only on cy (cy rides the ACT
            # HWDGE queue so the sync sequencer's first issue is tile 0)
            cy_t = small.tile([RB, 1], F32)
            nc.scalar.dma_start(out=cy_t[:, :], in_=cy_ext[:, :])
            sq = small.tile([RB, 1], F32)
            nc.vector.tensor_tensor(out=sq[:, :], in0=cy_t[:, :], in1=cy_t[:, :],
                                    op=ALU.mult)
            om = small.tile([RB, 1], F32)
            nc.vector.tensor_scalar(out=om[:, :], in0=sq[:, :], scalar1=-1.0,
                                    scalar2=1.0, op0=ALU.mult, op1=ALU.add)
            lnom = small.tile([RB, 1], F32)
            nc.scalar.activation(lnom[:, :], om[:, :], AF.Ln)
            rt = small.tile([RB, 1], F32)  # sqrt(om) = exp(0.5*ln(om)):
            nc.scalar.activation(rt[:, :], lnom[:, :], AF.Exp, scale=0.5)
            ca = small.tile([RB, 1], F32)
            nc.vector.tensor_scalar_mul(ca[:, :], cy_t[:, :], cos_m)
            cb = small.tile([RB, 1], F32)
            nc.vector.tensor_scalar_mul(cb[:, :], rt[:, :], sin_m)
            cm = small.tile([RB, 1], F32)
            nc.vector.tensor_tensor(out=cm[:, :], in0=ca[:, :], in1=cb[:, :],
                                    op=ALU.subtract)
            tn = small.tile([RB, 1], F32)  # SCALE * cos(acos(cy)+m)
            nc.vector.tensor_scalar_mul(tn[:, :], cm[:, :], SCALE)
            en = small.tile([RB, 1], F32)
            nc.scalar.activation(en[:, :], tn[:, :], AF.Exp)
            eo = small.tile([RB, 1], F32)
            nc.scalar.activation(eo[:, :], cy_t[:, :], AF.Exp, scale=SCALE)
            delta = small.tile([RB, 1], F32)  # exp(new) - exp(old) per row
            nc.vector.tensor_tensor(out=delta[:, :], in0=en[:, :], in1=eo[:, :],
                                    op=ALU.subtract)
            # fold sum_r(-tn_r) into the loss accumulator now (PSUM accumulate
            # needs no extra sems between matmuls)
            # Ln's spline LUT cannot represent inputs ~1e30, so the log is
            # evaluated on s * 2^-104 (exact power-of-2 scaling in the ACT
            # affine stage); the +104*ln2 compensation rides along in tnshift.
            tnshift = small.tile([RB, 1], F32)
            nc.vector.tensor_scalar(out=tnshift[:, :], in0=tn[:, :], scalar1=1.0,
                                    scalar2=-104.0 * math.log(2.0), op0=ALU.mult,
                                    op1=ALU.add)
            nc.tensor.matmul(acc_psum[:, :], lhsT=tnshift[:, :], rhs=negones[:, :],
                             start=False, stop=False, skip_group_check=True)
            # pre-load s_psum with delta so the collapse matmuls land on top
            s_psum = psum_pool.tile([RB, 1], F32)
            nc.tensor.matmul(s_psum[:, :], lhsT=id64[:, :], rhs=delta[:, :],
                             start=True, stop=False, skip_group_check=True)

            # ---- main stream: exp(SCALE*x) with per-partition accumulate.
            # DMA granularity (FULL_TILES/NARROW_TILES) pipelines the loads;
            # group granularity merges bulk tiles pairwise to halve ACT's
            # fixed per-instruction cost.
            xbig = stream.tile([128, HALF], F32)
            xd = stream.tile([120, DCHUNK], F32)
            ngf = len(FULL_GROUPS)
            ngn = len(NARROW_TILES)
            stats = small.tile([128, ngf], F32)
            statsn = small.tile([120, ngn], F32)
            statsd = small.tile([120, 1], F32)

            # tile 0 first so ACT starts early, then the donor block (its exp
            # and collapse matmul retire long before the tail)
            c0 = 0
            t = 0
            g = 0
            first = True
            for gsz in FULL_GROUPS:
                g0 = c0
                for _ in range(gsz):
                    ft = FULL_TILES[t]
                    nc.sync.dma_start(out=xbig[:, c0:c0 + ft],
                                      in_=x[:, c0:c0 + ft])
                    c0 += ft
                    t += 1
                    if first:
                        # donor DMA: 120 descriptors -> engines 64-78 only
                        nc.sync.dma_start(out=xd[:, :], in_=donor_src[:, :])
                        first = False
                nc.scalar.activation(xbig[:, g0:c0], xbig[:, g0:c0], AF.Exp,
                                     scale=SCALE, accum_out=stats[:, g:g + 1])
                if g == 0:
                    nc.scalar.activation(xd[:, :], xd[:, :], AF.Exp,
                                         scale=SCALE, accum_out=statsd[:, :])
                g += 1

            # narrow region: columns [FULLW, HALF) of partitions 0-119
            for gn, nt in enumerate(NARROW_TILES):
                nc.sync.dma_start(out=xbig[0:120, c0:c0 + nt],
                                  in_=x[0:120, c0:c0 + nt])
                nc.scalar.activation(xbig[0:120, c0:c0 + nt],
                                     xbig[0:120, c0:c0 + nt], AF.Exp,
                                     scale=SCALE,
                                     accum_out=statsn[:, gn:gn + 1])
                c0 += nt
            assert c0 == HALF

            # ---- per-partition totals, then collapse to per-row sums
            # (accumulated onto the delta preload in s_psum). The reduces run
            # as ACT Copies with accum_out so they follow the exps on the same
            # engine with no cross-engine semaphore hop. Donor collapse went
            # through edonor at ~13us; the two tail matmuls each wait only on
            # their tvec.
            nc.tensor.matmul(s_psum[:, :], lhsT=edonor[:, :], rhs=statsd[:, :],
                             start=False, stop=False, skip_group_check=True)
            tvecn = small.tile([120, 1], F32)
            statsn_cp = small.tile([120, ngn], F32)
            nc.scalar.activation(statsn_cp[:, :], statsn[:, :], AF.Copy,
                                 accum_out=tvecn[:, :])
            nc.tensor.matmul(s_psum[:, :], lhsT=emat[0:120, :], rhs=tvecn[:, :],
                             start=False, stop=False, skip_group_check=True)
            tvec = small.tile([128, 1], F32)
            stats_cp = small.tile([128, ngf], F32)
            nc.scalar.activation(stats_cp[:, :], stats[:, :], AF.Copy,
                                 accum_out=tvec[:, :])
            nc.tensor.matmul(s_psum[:, :], lhsT=emat[:, :], rhs=tvec[:, :],
                             start=False, stop=True, skip_group_check=True)
            lse = small.tile([RB, 1], F32)
            nc.scalar.activation(lse[:, :], s_psum[:, :], AF.Ln, scale=2.0 ** -104)
            nc.tensor.matmul(acc_psum[:, :], lhsT=lse[:, :], rhs=ones[:, :],
                             start=False, stop=True, skip_group_check=True)

            partial_sb = small.tile([1, 1], F32)
            nc.scalar.copy(partial_sb[:, :], acc_psum[:, :])
            nc.sync.dma_start(out=out_ext[:, :], in_=partial_sb[:, :])

    nc.finalize()  # Bacc.compile(): reg alloc + split multi-sem waits for TRN2
    return nc


_NC = None


def kernel(costh: np.ndarray, label: np.ndarray) -> np.ndarray:
    global _NC
    costh = np.ascontiguousarray(np.asarray(costh, dtype=np.float32))
    label = np.asarray(label).astype(np.int64)
    assert costh.shape == (B, C) and label.shape == (B,)

    cy = costh[np.arange(B), label].astype(np.float32)  # host gather of c_y

    if _NC is None:
        _NC = _build()

    in_maps = []
    for i in range(N_CORES):
        in_maps.append({
            "costh": np.ascontiguousarray(costh[i * RB:(i + 1) * RB]),
            "cy": np.ascontiguousarray(cy[i * RB:(i + 1) * RB].reshape(RB, 1)),
        })

    res = run_bass_kernel_spmd(_NC, in_maps, list(range(N_CORES)))
    out = np.float32(
        sum(float(res.results[i]["out"][0, 0]) for i in range(N_CORES)) / B)
    kernel.last_exec_time_ns = res.exec_time_ns
    return out
